# revision 2
# baseline (speedup 1.0000x reference)
"""GatedCrossAttention kernel for 8 Trainium2 NeuronCores.

Sharding: 8 cores = 4 batches x 2 T-halves. Core i handles batch i//2 and
query rows [i%2 * T/2, (i%2+1) * T/2). Each core receives only its batch's
key slice (halves host->device traffic vs full replication) and computes the
full fused gated-cross-attention for its shard with no collectives.
Matmuls run with bf16 operands and fp32 accumulation; softmax/gating in fp32.
"""

import numpy as np
import jax
import jax.numpy as jnp

EMBED_DIM = 1024
ZDIM = 128
N_CORES = 8
BF16 = jnp.bfloat16
F32 = jnp.float32


def _mm(a, b, pat):
    return jnp.einsum(pat, a.astype(BF16), b.astype(BF16),
                      preferred_element_type=F32)


def _compute(query, key, Wq, bq, Wk, bk, Wv, bv, Wh, bh, gamma, beta):
    E, Z = EMBED_DIM, ZDIM
    scaling = Z ** (-0.5)
    base = _mm(query, Wq, 'tbe,fe->tbf') + bq
    u = jax.nn.sigmoid(base[..., :E])
    rq = jax.nn.silu(base[..., E:])
    r = rq[..., :E]
    q = rq[..., E:] * gamma[0] + beta[0]
    k = jax.nn.silu(_mm(key, Wk, 'sbe,ze->sbz') + bk) * gamma[1] + beta[1]
    v = jax.nn.silu(_mm(key, Wv, 'sbe,fe->sbf') + bv)
    qk = _mm(q * scaling, k, 'tbz,sbz->bts')
    attn = jax.nn.softmax(qk, axis=-1)
    h = _mm(attn, v, 'bts,sbf->tbf')
    h = jnp.tanh(_mm(h * r, Wh, 'tbe,fe->tbf') + bh)
    return query + u * (h - query)


_pmapped = jax.pmap(
    _compute,
    in_axes=(0, 0) + (None,) * 10,
)


def kernel(**inputs) -> np.ndarray:
    query = np.asarray(inputs["query"], np.float32)
    key = np.asarray(inputs["key"], np.float32)
    T, B, E = query.shape
    S = key.shape[0]
    half = T // 2
    # core i -> batch i//2, T-half i%2
    # q_sh: [8, half, 1, E]; key_sh: [8, S, 1, E]
    q_sh = np.ascontiguousarray(
        query.transpose(1, 0, 2).reshape(B, 2, half, 1, E).reshape(8, half, 1, E)
    )
    key_b = np.ascontiguousarray(key.transpose(1, 0, 2))  # [B, S, E]
    key_sh = np.repeat(key_b[:, None], 2, axis=1).reshape(8, S, 1, E)
    out = _pmapped(
        q_sh,
        key_sh,
        jnp.asarray(inputs["Wq"], F32),
        jnp.asarray(inputs["bq"], F32),
        jnp.asarray(inputs["Wk"], F32),
        jnp.asarray(inputs["bk"], F32),
        jnp.asarray(inputs["Wv"], F32),
        jnp.asarray(inputs["bv"], F32),
        jnp.asarray(inputs["Wh"], F32),
        jnp.asarray(inputs["bh"], F32),
        jnp.asarray(inputs["gamma"], F32),
        jnp.asarray(inputs["beta"], F32),
    )
    out = np.asarray(out)  # [8, half, 1, E]
    out = out.reshape(B, 2, half, E).reshape(B, T, E).transpose(1, 0, 2)
    return np.ascontiguousarray(out).astype(np.float32)


# revision 3
# speedup vs baseline: 1.1175x; 1.1175x over previous
"""GatedCrossAttention kernel for 8 Trainium2 NeuronCores.

Sharding: the query/time dimension T (=2048) is split into 8 shards of 256;
each core runs the full fused gated-cross-attention for its T-shard across
all batches (key/weights replicated — k/v projections are cheap relative to
the T-dependent work, and this avoids any collective).
"""

import numpy as np
import jax
import jax.numpy as jnp

EMBED_DIM = 1024
ZDIM = 128
N_CORES = 8


def _compute(query, key, Wq, bq, Wk, bk, Wv, bv, Wh, bh, gamma, beta):
    E, Z = EMBED_DIM, ZDIM
    scaling = Z ** (-0.5)
    base = jnp.einsum('tbe,fe->tbf', query, Wq) + bq
    u = jax.nn.sigmoid(base[..., :E])
    rq = jax.nn.silu(base[..., E:])
    r = rq[..., :E]
    q = rq[..., E:] * gamma[0] + beta[0]
    k = jax.nn.silu(jnp.einsum('sbe,ze->sbz', key, Wk) + bk) * gamma[1] + beta[1]
    v = jax.nn.silu(jnp.einsum('sbe,fe->sbf', key, Wv) + bv)
    qk = jnp.einsum('tbz,sbz->bts', q * scaling, k)
    attn = jax.nn.softmax(qk, axis=-1)
    h = jnp.einsum('bts,sbf->tbf', attn, v)
    h = jnp.tanh(jnp.einsum('tbe,fe->tbf', h * r, Wh) + bh)
    return query + u * (h - query)


_pmapped = jax.pmap(
    _compute,
    in_axes=(0,) + (None,) * 11,
)


def kernel(**inputs) -> np.ndarray:
    query = np.asarray(inputs["query"], np.float32)
    T = query.shape[0]
    q_sh = query.reshape(N_CORES, T // N_CORES, *query.shape[1:])
    out = _pmapped(
        q_sh,
        jnp.asarray(inputs["key"], jnp.float32),
        jnp.asarray(inputs["Wq"], jnp.float32),
        jnp.asarray(inputs["bq"], jnp.float32),
        jnp.asarray(inputs["Wk"], jnp.float32),
        jnp.asarray(inputs["bk"], jnp.float32),
        jnp.asarray(inputs["Wv"], jnp.float32),
        jnp.asarray(inputs["bv"], jnp.float32),
        jnp.asarray(inputs["Wh"], jnp.float32),
        jnp.asarray(inputs["bh"], jnp.float32),
        jnp.asarray(inputs["gamma"], jnp.float32),
        jnp.asarray(inputs["beta"], jnp.float32),
    )
    return np.asarray(out).reshape(T, *query.shape[1:]).astype(np.float32)
